# revision 22
# baseline (speedup 1.0000x reference)
"""Trainium2 Bass kernel for BiLevelRoutingAttention (nn_BiLevelRoutingAttention_66907000537867).

Sharding: one attention head per NeuronCore (8 heads / 8 cores).

Per core, three phases (all-bf16 matmuls, image-order pixel layout):
  phase 1: qkv projection in bf16 over 7 window-row tiles (16 image rows
           each). q evacuated image-order, k permuted to window order in
           the evacuation AP, v evacuated straight into the padded-image
           tensor vs4 (band dy0) and block-transposed to pixel-major (DVE
           stream transpose) for the attn@V stationaries. PE row-tiling
           band replication via per-tile SBUF-SBUF DMAs.
  phase 2: per region (49): QK^T as 4 concurrent 32-row PE tiles ->
           exp on ScalarE as 2x [128,1024] ACTIVATEs (scale fused) ->
           attn@V with a ones column producing softmax denominators in
           row 32 -> per-window-row scatter/reciprocal/gather (spreads
           the 1792 denominators over 128 DVE lanes) -> K=1 broadcast
           matmul + DVE normalize into out_att, lagged 9 regions behind.
           ScalarE is the bottleneck (~2.4us/region); filler matmuls keep
           the PE HAM clock-gate at full speed (it otherwise never sees a
           full busy window and the whole phase runs at 1.2 GHz).
  phase 3: output projection with lepe folded in, as 3 full-contraction
           matmuls per (group, half): vs4's four 32-partition bands hold
           the three dy-shifted v copies + the normalized attention map,
           so contraction over 128 partitions covers taps (dy0,dy1,dy2,
           attn) at once; the 3 matmuls are the dx shifts. Output bf16;
           host sums the 8 per-head partials.

Host: region routing (top-k is metadata; the mean commutes with the linear
qkv layer), per-head weight slicing + lepe fold into dy-stacked
stationaries, bf16 casts, final partial-sum + constant bias row.
"""

import numpy as np
import ml_dtypes

import concourse.bass as bass
import concourse.bacc as bacc
import concourse.mybir as mybir
import concourse.tile as tile
from concourse.bass_utils import run_bass_kernel_spmd

F32 = mybir.dt.float32
BF16 = mybir.dt.bfloat16
AF = mybir.ActivationFunctionType

DIM, QK, HEADS, NWIN, TOPK = 256, 256, 8, 7, 4
H = W = 112
P2 = NWIN * NWIN          # 49 regions
W2 = 256                  # pixels per region (16x16)
NPIX = H * W              # 12544
HD = 32                   # per-head dim
SCALE = QK ** (-0.5)      # 1/16
PW = 114                  # padded image width
TW = 1792                 # phase-1 tile = one window row (16 image rows)
NT = 7                    # phase-1 tiles
NG = 28                   # phase-3 pixel groups (4 image rows each)
N3 = 448                  # pixels per phase-3 group
LAG = 9                   # phase-2 normalize lag (window-row recip batch)
NFILL = 6                 # phase-2 PE warmth fillers per region

_cache = {}


def _build(top_idx, debug=False):
    nc = bacc.Bacc()
    xT_d = nc.declare_dram_parameter("xT", [DIM, NPIX], BF16, isOutput=False)
    wqkv_d = nc.declare_dram_parameter("wqkv", [DIM, 96], BF16, isOutput=False)
    bqkv_d = nc.declare_dram_parameter("bqkv", [96, 1], F32, isOutput=False)
    wt_d = nc.declare_dram_parameter("wt", [128, 768], BF16, isOutput=False)
    out_d = nc.declare_dram_parameter("out", [DIM, NPIX], BF16, isOutput=True)

    with tile.TileContext(nc) as tc, tc.tile_pool(name="persist", bufs=1) as persist:
        # ---- persistent SBUF ----
        w_sb = persist.tile([128, 192], BF16)         # qkv weights, 2 cin chunks
        bqkv_sb = persist.tile([96, 1], F32)
        qq = persist.tile([128, NPIX], BF16)          # q image-order, 4 bands
        kk = persist.tile([128, NPIX], BF16)          # k window-order, 4 bands
        # vs4: padded-image planes. p64-95: v (dy0) written by phase 1;
        # p0-31 / p32-63: v shifted up 1 / 2 rows; p96-127: attention map.
        vs4 = persist.tile([128, PW * PW], BF16)
        vt = persist.tile([128, NPIX], BF16)          # stream-transposed v staging
        v_aug = persist.tile([128, 98, 34], BF16)     # pixel-major v + ones col 32
        u_sb = persist.tile([33, NPIX], BF16)         # unnormalized attn out + denom
        drw = persist.tile([1, NPIX], BF16)           # reciprocal denominators
        out_att = persist.tile([32, NPIX], BF16)      # normalized attn (image order)
        wt_sb = persist.tile([128, 768], BF16)        # dy-stacked proj taps
        ones_sb = persist.tile([1, 128], BF16)

        nc.sync.dma_start(out=w_sb[:, 0:96], in_=wqkv_d[0:128, :])
        nc.sync.dma_start(out=w_sb[:, 96:192], in_=wqkv_d[128:256, :])
        nc.sync.dma_start(out=bqkv_sb, in_=bqkv_d[:, :])
        nc.sync.dma_start(out=wt_sb, in_=wt_d[:, :])
        nc.vector.memset(ones_sb, 1.0)
        nc.vector.memset(v_aug[:, :, 32:33], 1.0)
        nc.gpsimd.memset(vs4, 0.0)  # borders stay zero; interiors overwritten

        vs4_v = vs4.rearrange("p (r c) -> p r c", c=PW)
        qq_v = qq.rearrange("p (r c) -> p r c", c=W)

        # ---- phase 1: qkv projection ----
        with (
            tc.tile_pool(name="xt", bufs=2) as xtp,
            tc.tile_pool(name="qkv_ps", bufs=2, space="PSUM") as qkvps,
        ):
            for t in range(NT):
                n0 = TW * t
                xt0 = xtp.tile([128, TW], BF16, tag="xt0")
                xt1 = xtp.tile([128, TW], BF16, tag="xt1")
                nc.sync.dma_start(out=xt0, in_=xT_d[0:128, n0:n0 + TW])
                nc.sync.dma_start(out=xt1, in_=xT_d[128:256, n0:n0 + TW])
                ps = qkvps.tile([96, TW], F32, tag="qkv")
                for blk in range(4):
                    c0, cw = 512 * blk, (512 if blk < 3 else 256)
                    nc.tensor.matmul(ps[:, c0:c0 + cw], w_sb[:, 0:96],
                                     xt0[:, c0:c0 + cw], start=True, stop=False)
                    nc.tensor.matmul(ps[:, c0:c0 + cw], w_sb[:, 96:192],
                                     xt1[:, c0:c0 + cw], start=False, stop=True)
                # evacuations (engines alternate). k is permuted to window
                # order in the AP so QK^T stationaries are 1D-contiguous.
                kdst = kk[32:64, n0:n0 + TW].rearrange("p (w a b) -> p w a b",
                                                       a=16, b=16)
                ksrc = ps[32:64, :].rearrange("p (a w b) -> p w a b",
                                              w=NWIN, b=16)
                vdst = vs4_v[64:96, 16 * t + 1:16 * t + 17, 1:113]
                psv = ps[64:96, :].rearrange("p (a b) -> p a b", b=W)
                if t % 2 == 0:
                    nc.vector.tensor_scalar_add(qq[0:32, n0:n0 + TW], ps[0:32, :],
                                                bqkv_sb[0:32, 0:1])
                    nc.vector.tensor_scalar_add(kdst, ksrc, bqkv_sb[32:64, 0:1])
                    nc.scalar.activation(vdst, psv, AF.Identity,
                                         bias=bqkv_sb[64:96, 0:1])
                else:
                    nc.scalar.activation(qq[0:32, n0:n0 + TW], ps[0:32, :],
                                         AF.Identity, bias=bqkv_sb[0:32, 0:1])
                    nc.scalar.activation(kdst, ksrc, AF.Identity,
                                         bias=bqkv_sb[32:64, 0:1])
                    nc.vector.tensor_scalar_add(vdst, psv, bqkv_sb[64:96, 0:1])
                # v -> pixel-major via DVE 32x32 stream transpose (window order)
                vsrc = vs4_v[64:96, 16 * t + 1:16 * t + 17, 1:113]
                vsrc = vsrc.rearrange("p a (w b) -> p w a b", b=16)
                nc.vector.transpose(vt[64:96, n0:n0 + TW], vsrc)
                # band replication + v_aug regroup (per tile; Sync has slack)
                for b in (1, 2, 3):
                    nc.sync.dma_start(out=qq[32 * b:32 * b + 32, n0:n0 + TW],
                                      in_=qq[0:32, n0:n0 + TW])
                for b in (0, 2, 3):
                    nc.sync.dma_start(out=kk[32 * b:32 * b + 32, n0:n0 + TW],
                                      in_=kk[32:64, n0:n0 + TW])
                vtv = vt.rearrange("p (c j) -> p c j", j=32)
                for a in range(4):
                    nc.sync.dma_start(
                        out=v_aug[32 * a:32 * a + 32, 14 * t:14 * t + 14, 0:32],
                        in_=vtv[64:96, 56 * t + a:56 * t + 56:4, :])

        tc.strict_bb_all_engine_barrier()

        # ---- phase 2: attention (software-pipelined over regions) ----
        with (
            tc.tile_pool(name="at_ps", bufs=1, space="PSUM") as atps,
            tc.tile_pool(name="av_ps", bufs=2, space="PSUM") as avps,
            tc.tile_pool(name="bc_ps", bufs=2, space="PSUM") as bcps,
            tc.tile_pool(name="exp", bufs=3) as expp,
            tc.tile_pool(name="dsc", bufs=2) as dscp,
        ):
            # v shifted-band replication (needed by phase 3 only)
            nc.sync.dma_start(out=vs4[0:32, 0:PW * PW - PW],
                              in_=vs4[64:96, PW:PW * PW])
            nc.sync.dma_start(out=vs4[32:64, 0:PW * PW - 2 * PW],
                              in_=vs4[64:96, 2 * PW:PW * PW])

            def filler(bc, n):
                # junk matmuls into the (later overwritten) bc tile keep the
                # PE HAM activity monitor busy so it holds the 2.4 GHz clock.
                for i in range(n):
                    nc.tensor.matmul(bc, wt_sb[0:32, 0:128],
                                     qq[0:32, 256 * i:256 * i + 256],
                                     start=True, stop=True,
                                     tile_position=(0, 0))

            chunk_list = [[2 * g + jj for g in top_idx[r] for jj in (0, 1)]
                          for r in range(P2)]
            st = {}
            for r in range(P2 + LAG):
                # -- 32x128-mode PE group: QK^T_r, fillers, bc_{r-LAG}
                bc = bcps.tile([128, W2], F32, tag="bc")
                if r == 0:
                    filler(bc, 30)  # warmup burst (~3.5us busy)
                if r < P2:
                    chunks = chunk_list[r]
                    wr, wc = divmod(r, NWIN)
                    atA = atps.tile([128, 1024], F32, tag="atA")
                    atB = atps.tile([128, 1024], F32, tag="atB")
                    for j in (0, 2, 4, 6, 1, 3, 5, 7):
                        c = chunks[j]
                        b = j // 2
                        at = atA if b < 2 else atB
                        col = 512 * (b % 2) + 256 * (j % 2)
                        nc.tensor.matmul(
                            at[:, col:col + 256],
                            kk[32 * b:32 * b + 32, 128 * c:128 * c + 128],
                            qq_v[32 * b:32 * b + 32, 16 * wr:16 * wr + 16,
                                 16 * wc:16 * wc + 16],
                            start=True, stop=True, tile_position=(32 * b, 0))
                filler(bc, NFILL)
                if LAG <= r:
                    rn = r - LAG
                    nc.tensor.matmul(bc, ones_sb, drw[0:1, W2 * rn:W2 * rn + W2],
                                     start=True, stop=True)
                # -- ScalarE: exp
                if r < P2:
                    ex = expp.tile([128, 2048], BF16, tag="ex")
                    nc.scalar.activation(ex[:, 0:1024], atA, AF.Exp, scale=SCALE)
                    nc.scalar.activation(ex[:, 1024:2048], atB, AF.Exp,
                                         scale=SCALE)
                    st[r] = (chunks, ex)
                # -- 128-mode PE group: attn@V of r-1
                if 1 <= r <= P2:
                    chunks, ex = st.pop(r - 1)
                    avT = avps.tile([33, W2], F32, tag="av")
                    for j in range(8):
                        nc.tensor.matmul(avT, v_aug[:, chunks[j], 0:33],
                                         ex[:, 256 * j:256 * j + 256],
                                         start=(j == 0), stop=(j == 7))
                    # stash unnormalized out + denominator row
                    nc.vector.tensor_copy(u_sb[:, W2 * (r - 1):W2 * r], avT)
                # -- per-window-row denominator reciprocal (128 lanes)
                if 1 <= r <= P2 and (r - 1) % NWIN == NWIN - 1:
                    mwr = (r - 1) // NWIN
                    n0 = TW * mwr
                    dsc = dscp.tile([128, 16], BF16, tag="dsc")
                    dsc2 = dscp.tile([128, 16], BF16, tag="dsc2")
                    nc.sync.dma_start(out=dsc[:, 0:14], in_=u_sb[32:33, n0:n0 + TW])
                    with nc.allow_low_precision(reason="bf16 softmax denom"):
                        nc.vector.reciprocal(dsc2[:, 0:14], dsc[:, 0:14])
                    nc.sync.dma_start(out=drw[0:1, n0:n0 + TW], in_=dsc2[:, 0:14])
                # -- DVE normalize of r-LAG into out_att
                if LAG <= r:
                    rn = r - LAG
                    mwr, mwc = divmod(rn, NWIN)
                    dst = out_att.rearrange("p (a c) -> p a c", c=W)
                    dst = dst[0:32, 16 * mwr:16 * mwr + 16, 16 * mwc:16 * mwc + 16]
                    uv = u_sb[0:32, W2 * rn:W2 * rn + W2]
                    uv = uv.rearrange("p (a b) -> p a b", b=16)
                    bcv = bc[0:32, :].rearrange("p (a b) -> p a b", b=16)
                    nc.vector.tensor_mul(dst, uv, bcv)
                    # completed window row -> attention plane of vs4 (band 3)
                    if rn % NWIN == NWIN - 1:
                        n0 = TW * mwr
                        nc.sync.dma_start(
                            out=vs4_v[96:128, 16 * mwr:16 * mwr + 16, 0:112],
                            in_=out_att[0:32, n0:n0 + TW])

        tc.strict_bb_all_engine_barrier()

        # ---- phase 3: output projection + folded lepe (dy-stacked K=128) ----
        with (
            tc.tile_pool(name="o_ps", bufs=1, space="PSUM") as ops,
            tc.tile_pool(name="evsb", bufs=2) as evp,
        ):
            for q in range(NG // 4):
                ots = {(h, pp): ops.tile([128, 1024], F32, tag=f"o{h}{pp}",
                                         name=f"ot{h}{pp}")
                       for h in range(2) for pp in range(2)}
                for dx in range(3):
                    for h in range(2):
                        for gi in range(4):
                            g = 4 * q + gi
                            ot = ots[(h, gi // 2)]
                            col = 512 * (gi % 2)
                            nc.tensor.matmul(
                                ot[:, col:col + 448],
                                wt_sb[:, 128 * (2 * dx + h):128 * (2 * dx + h) + 128],
                                vs4_v[:, 4 * g:4 * g + 4, dx:dx + 112],
                                start=(dx == 0), stop=(dx == 2))
                for pp in range(2):
                    for h in range(2):
                        ev = evp.tile([128, 896], BF16, tag=f"ev{h}{pp}",
                                      name=f"ev{h}{pp}")
                        src = ots[(h, pp)].rearrange(
                            "p (i c) -> p i c", c=512)[:, :, 0:448]
                        dstv = ev.rearrange("p (i c) -> p i c", c=448)
                        if (pp + h) % 2 == 0:
                            nc.vector.tensor_copy(dstv, src)
                        else:
                            nc.scalar.copy(dstv, src)
                        g0 = 4 * q + 2 * pp
                        nc.sync.dma_start(
                            out=out_d[128 * h:128 * h + 128,
                                      N3 * g0:N3 * (g0 + 2)],
                            in_=ev)

        if debug:
            tc.strict_bb_all_engine_barrier()
            dbg = {
                "dbg_qq": qq, "dbg_kk": kk, "dbg_vaug": v_aug,
                "dbg_vs4": vs4, "dbg_outatt": out_att, "dbg_u": u_sb,
            }
            for name, t in dbg.items():
                sh = [t.shape[0], int(np.prod(t.shape[1:]))]
                d = nc.declare_dram_parameter(name, sh, t.dtype, isOutput=True)
                nc.sync.dma_start(out=d[:, :], in_=t.rearrange(
                    "p ... -> p (...)") if len(t.shape) > 2 else t[:, :])
    nc.compile()
    return nc


def _host_prep(x, w_qkv, b_qkv):
    xT = np.ascontiguousarray(
        x.reshape(NPIX, DIM).T).astype(ml_dtypes.bfloat16)
    xmean = x.reshape(NWIN, 16, NWIN, 16, DIM).mean((1, 3)).reshape(P2, DIM)
    q_win = xmean @ w_qkv[:, :QK] + b_qkv[:QK]
    k_win = xmean @ w_qkv[:, QK:2 * QK] + b_qkv[QK:2 * QK]
    logit = (q_win * SCALE) @ k_win.T
    top_idx = np.argsort(-logit, axis=-1, kind="stable")[:, :TOPK]
    return xT, top_idx


def _in_maps(x, w_qkv, b_qkv, w_o, lepe_w):
    xT, top_idx = _host_prep(x[0], w_qkv, b_qkv)
    lw = lepe_w[:, :, 0, :]  # [3,3,256]
    maps = []
    for h in range(HEADS):
        sl = slice(h * HD, (h + 1) * HD)
        wqkv_h = np.concatenate(
            [w_qkv[:, :QK][:, sl], w_qkv[:, QK:2 * QK][:, sl],
             w_qkv[:, 2 * QK:][:, sl]], axis=1).astype(ml_dtypes.bfloat16)
        bqkv_h = np.concatenate(
            [b_qkv[:QK][sl], b_qkv[QK:2 * QK][sl], b_qkv[2 * QK:][sl]])
        w_o_h = w_o[sl, :]  # [32, 256]
        # dy-stacked stationaries: per (dx, half), [128, 128]. Row blocks
        # match the vs4 band contents: p0-31 = dy1-shifted v, p32-63 = dy2,
        # p64-95 = dy0 (unshifted), p96-127 = attn (w_o, dx=0 only).
        row_of_dy = {1: 0, 2: 32, 0: 64}
        wt4 = np.zeros((128, 3, 2, 128), np.float32)
        for dx in range(3):
            for dy in range(3):
                blk = lw[dy, dx, sl][:, None] * w_o_h  # [32, 256]
                r0 = row_of_dy[dy]
                for hh in range(2):
                    wt4[r0:r0 + 32, dx, hh] = blk[:, 128 * hh:128 * hh + 128]
            if dx == 0:
                for hh in range(2):
                    wt4[96:128, 0, hh] = w_o_h[:, 128 * hh:128 * hh + 128]
        maps.append({
            "xT": xT,
            "wqkv": np.ascontiguousarray(wqkv_h),
            "bqkv": np.ascontiguousarray(bqkv_h[:, None]).astype(np.float32),
            "wt": np.ascontiguousarray(wt4.reshape(128, 768)).astype(
                ml_dtypes.bfloat16),
        })
    return maps, top_idx


def kernel(x, w_qkv, b_qkv, w_o, b_o, lepe_w, lepe_b):
    x = np.asarray(x, np.float32)
    w_qkv = np.asarray(w_qkv, np.float32)
    b_qkv = np.asarray(b_qkv, np.float32)
    w_o = np.asarray(w_o, np.float32)
    b_o = np.asarray(b_o, np.float32)
    lepe_w = np.asarray(lepe_w, np.float32)
    lepe_b = np.asarray(lepe_b, np.float32)

    maps, top_idx = _in_maps(x, w_qkv, b_qkv, w_o, lepe_w)
    key = top_idx.tobytes()
    if key not in _cache:
        _cache[key] = _build(top_idx)
    nc = _cache[key]

    res = run_bass_kernel_spmd(nc, maps, list(range(HEADS))).results
    total = np.zeros((DIM, NPIX), np.float32)
    for h in range(HEADS):
        total += np.asarray(res[h]["out"], np.float32)
    b_all = lepe_b @ w_o + b_o
    out = total.T + b_all
    return out.reshape(1, H, W, DIM).astype(np.float32)
